# revision 6
# baseline (speedup 1.0000x reference)
"""Trainium2 Bass kernel for causal multi-head attention (fp16 rewrite).

Problem: B=2, S=2048, D=2048, H=16 heads (HD=128), fp32 in/out, causal.
Sharding: 8 cores = 2 batches (data parallel) x 4 head-groups (tensor
parallel, 4 heads each). Each core computes Q/K/V projections for its
head slice, causal attention, and a partial out-projection; the host
sums the 4 partials per batch (in fp32) and adds the output bias.

Device layout notes:
  - fp16 operands everywhere on the PE (1 cycle/row with NO minimum
    moving-dim threshold, unlike fp32r's N>=256), fp32 PSUM accumulation.
    fp16's 10 mantissa bits keep the global rel err in the few-1e-4 range.
  - All weight tiles persist in SBUF (loaded once, 6 MB fp16); x chunks
    are double-buffered and prefetched so the PE never waits on DMA at
    chunk boundaries.
  - Scores are computed transposed (scores^T[k, q]); softmax runs
    unnormalized with a constant -6 bias folded into the exp so fp16 pt
    can't overflow (exp(s/sqrt(128) - 6) <= ~e); the bias cancels in the
    normalization exactly.
  - Softmax denominators accumulate on the PE via a ones-vector matmul
    (cheapest engine for partition reductions at 2.4 GHz); the
    reciprocal is broadcast across partitions by a rank-1 PE matmul into
    a borrowed out-proj PSUM slot (a GPSIMD partition_broadcast measured
    no faster on real HW and sits on the per-head critical path).
  - Scores matmuls are emitted one step ahead of each tile's den/AV
    matmuls (including across head/chunk boundaries) so the in-order PE
    queue always has work while the exp runs on the Act engine.
  - PSUM: phase 2 uses exactly 8 banks: scores x2, ctx x2, den x2,
    out-proj x2. Denominators share banks pairwise (partitions 0/64),
    so no per-head den-bank recycling stall exists. Output row blocks
    leave as one contiguous 4KB-row DMA per q-tile (streamed per-oc for
    the final tile to overlap the drain).
"""

import sys

if "/opt/trn_rl_repo" not in sys.path:
    sys.path.insert(0, "/opt/trn_rl_repo")

import numpy as np

import concourse.bacc as bacc
import concourse.mybir as mybir
import concourse.tile as tile
from concourse.bass_utils import run_bass_kernel_spmd
from concourse.masks import make_upper_triangular

B, S, D, H = 2, 2048, 2048, 16
HD = 128                 # head dim
NCORES = 8
HPC = 4                  # heads per core
DC = HPC * HD            # 512: per-core projection width
CT = D // 128            # 16 contraction tiles
QT = S // 512            # 4 query chunks of 512
ST = S // 128            # 16 seq tiles of 128
SCALE = 1.0 / float(np.sqrt(HD))
EBIAS = -6.0             # constant max-sub shift; cancels in normalization
F32 = mybir.dt.float32
F16 = mybir.dt.float16
EXP = mybir.ActivationFunctionType.Exp

_BUILT = None


def _build(cfg=None, reps=1):
    cfg = cfg or {}
    PTB = cfg.get("ptb", 4)    # p^T tile bufs
    SCB = cfg.get("scb", 2)    # scores psum bufs
    CPB = cfg.get("cpb", 2)    # ctx psum bufs
    DNB = cfg.get("dnb", 1)    # denom psum bufs (per A/B tag: 2 banks total)
    PPB = cfg.get("ppb", 2)    # proj psum bufs
    OPB = cfg.get("opb", 2)    # out-proj psum bufs
    nc = bacc.Bacc(trn_type="TRN2", target_bir_lowering=False)
    xT_d = nc.dram_tensor("xT", [D, S], F16, kind="ExternalInput")
    wqT_d = nc.dram_tensor("wqT", [D, DC], F16, kind="ExternalInput")
    wkT_d = nc.dram_tensor("wkT", [D, DC], F16, kind="ExternalInput")
    wvT_d = nc.dram_tensor("wvT", [D, DC], F16, kind="ExternalInput")
    woT_d = nc.dram_tensor("woT", [DC, D], F16, kind="ExternalInput")
    out_d = nc.dram_tensor("out", [S, D], F16, kind="ExternalOutput")

    with tile.TileContext(nc) as tc:
      for _rep in range(reps):
        _p = f"r{_rep}_"
        with (
            tc.tile_pool(name=_p + "const", bufs=1) as cst,
            tc.tile_pool(name=_p + "persist", bufs=1) as pp,
        ):
            # upper-triangular (incl diagonal) 0/1 mask: allowed = k <= q
            tri = cst.tile([128, 128], F16, tag="tri", name="tri")
            make_upper_triangular(nc, tri[:], val=1.0, diag=True)
            ones_f = cst.tile([128, 1], F32, tag="ones_f", name="ones_f")
            nc.vector.memset(ones_f[:], 1.0)
            ones_col = cst.tile([128, 1], F16, tag="ones_col", name="ones_col")
            nc.vector.tensor_copy(ones_col[:], ones_f[:])
            ebias = cst.tile([128, 1], F32, tag="ebias", name="ebias")
            nc.vector.memset(ebias[:], EBIAS)
            ones_rf = cst.tile([1, 128], F32, tag="ones_rf", name="ones_rf")
            nc.vector.memset(ones_rf[:], 1.0)
            ones_row = cst.tile([1, 128], F16, tag="ones_row", name="ones_row")
            nc.vector.tensor_copy(ones_row[:], ones_rf[:])

            # persistent per-core tensors (partition dim x free dim):
            # qT/kT: per head [HD, S]; v: per s-tile [128, DC]; ctx^T per
            # (head, q-chunk) for fine-grained deps so the out-projection of
            # chunk qt can overlap attention of chunk qt+1.
            qTt = [pp.tile([128, S], F16, tag=f"qT{h}", name=f"qT{h}") for h in range(HPC)]
            kTt = [pp.tile([128, S], F16, tag=f"kT{h}", name=f"kT{h}") for h in range(HPC)]
            vt = [pp.tile([128, DC], F16, tag=f"v{s}", name=f"v{s}") for s in range(ST)]
            ctxt = [[pp.tile([128, 512], F16, tag=f"ctx{h}_{q}", name=f"ctx{h}_{q}")
                     for q in range(QT)] for h in range(HPC)]

            # ---------------- Phase 1: Q/K/V projections ----------------
            with (
                tc.tile_pool(name=_p + "wqkv", bufs=1) as wp,
                tc.tile_pool(name=_p + "xc", bufs=2) as xcp,
                tc.tile_pool(name=_p + "proj_psum", bufs=PPB, space="PSUM") as pps,
            ):
                # Load all weight tiles once; interleave chunk-0 x tiles with
                # the wq tiles so the first matmuls' operands arrive first.
                wq_t, wk_t, wv_t = [], [], []
                xcs = []
                for ct in range(CT):
                    xc = xcp.tile([128, 512], F16, tag=f"xc{ct}", name=f"xc_0_{ct}")
                    nc.sync.dma_start(
                        out=xc[:], in_=xT_d[ct * 128:(ct + 1) * 128, 0:512])
                    xcs.append(xc)
                    w = wp.tile([128, DC], F16, tag=f"wq{ct}", name=f"wq{ct}")
                    nc.sync.dma_start(out=w[:], in_=wqT_d[ct * 128:(ct + 1) * 128, :])
                    wq_t.append(w)
                for w_d, lst, nm in ((wkT_d, wk_t, "wk"), (wvT_d, wv_t, "wv")):
                    for ct in range(CT):
                        w = wp.tile([128, DC], F16, tag=f"{nm}{ct}", name=f"{nm}{ct}")
                        nc.sync.dma_start(out=w[:], in_=w_d[ct * 128:(ct + 1) * 128, :])
                        lst.append(w)

                for n in range(QT):  # s-chunks of 512
                    if n + 1 < QT:
                        # prefetch next chunk (double-buffered tags)
                        nxt = []
                        for ct in range(CT):
                            xc = xcp.tile([128, 512], F16, tag=f"xc{ct}",
                                          name=f"xc_{n + 1}_{ct}")
                            nc.sync.dma_start(
                                out=xc[:],
                                in_=xT_d[ct * 128:(ct + 1) * 128,
                                         (n + 1) * 512:(n + 2) * 512])
                            nxt.append(xc)

                    # Q^T and K^T: out[d-tile(=head) 128, s 512] accum over ct
                    for w_t, dst in ((wq_t, qTt), (wk_t, kTt)):
                        acc = [pps.tile([128, 512], F32, tag=f"acc{m}", name=f"acc_{n}_{m}")
                               for m in range(HPC)]
                        for ct in range(CT):
                            for m in range(HPC):
                                nc.tensor.matmul(
                                    acc[m][:],
                                    (w_t[ct][:, m * 128:(m + 1) * 128]),
                                    (xcs[ct][:]),
                                    start=(ct == 0),
                                    stop=(ct == CT - 1),
                                )
                        for m in range(HPC):
                            nc.vector.tensor_copy(
                                dst[m][:, n * 512:(n + 1) * 512], acc[m][:]
                            )

                    # V natural [s-tile 128, d 512]: lhsT = x^T chunk slice
                    accv = [pps.tile([128, 512], F32, tag=f"acc{ss}", name=f"accv_{n}_{ss}")
                            for ss in range(4)]
                    for ct in range(CT):
                        for ss in range(4):
                            nc.tensor.matmul(
                                accv[ss][:],
                                (xcs[ct][:, ss * 128:(ss + 1) * 128]),
                                (wv_t[ct][:]),
                                start=(ct == 0),
                                stop=(ct == CT - 1),
                            )
                    for ss in range(4):
                        nc.vector.tensor_copy(vt[n * 4 + ss][:], accv[ss][:])
                    if n + 1 < QT:
                        xcs = nxt

            # ------- Phase 2+3: causal attention with interleaved out-proj ----
            with (
                tc.tile_pool(name=_p + "ptp", bufs=PTB) as ptp,
                tc.tile_pool(name=_p + "rcp", bufs=2) as rcp,
                tc.tile_pool(name=_p + "rcb", bufs=2) as rcbp,
                tc.tile_pool(name=_p + "wo", bufs=1) as wop,
                tc.tile_pool(name=_p + "osb", bufs=3) as osp,
                tc.tile_pool(name=_p + "sc_ps", bufs=SCB, space="PSUM") as scp,
                tc.tile_pool(name=_p + "ctx_ps", bufs=CPB, space="PSUM") as cxp,
                tc.tile_pool(name=_p + "den_ps", bufs=DNB, space="PSUM") as dnp,
                tc.tile_pool(name=_p + "out_ps", bufs=OPB, space="PSUM") as ops,
            ):
                # preload all out-proj weight tiles (16KB/partition fp16)
                wots = {}
                for oc in range(4):
                    for i in range(HPC):
                        wo_t = wop.tile([128, 512], F16, tag=f"wo{oc}_{i}",
                                        name=f"wo_{oc}_{i}")
                        nc.sync.dma_start(
                            out=wo_t[:],
                            in_=woT_d[i * 128:(i + 1) * 128, oc * 512:(oc + 1) * 512],
                        )
                        wots[(oc, i)] = wo_t

                def emit_scores(qt, h, kt):
                    j = kt - 4 * qt
                    # Diagonal blocks: only q-cols >= 128j are unmasked;
                    # shrink the moving range (fp16 runs 1 cyc/row at any
                    # N, so N=128 is fine).
                    lo = 0 if j < 0 else j * 128
                    sc = scp.tile([128, 512], F32, tag="sc", name=f"sc_{h}_{qt}_{kt}")
                    nc.tensor.matmul(
                        sc[:, lo:],
                        (kTt[h][:, kt * 128:(kt + 1) * 128]),
                        (qTt[h][:, qt * 512 + lo:(qt + 1) * 512]),
                        start=True,
                        stop=True,
                    )
                    return sc, lo

                def next_tile(qt, h, kt):
                    nkt = 4 * qt + 4
                    if kt + 1 < nkt:
                        return (qt, h, kt + 1)
                    if h + 1 < HPC:
                        return (qt, h + 1, 0)
                    if qt + 1 < QT:
                        return (qt + 1, 0, 0)
                    return None

                pend = emit_scores(0, 0, 0)
                for qt in range(QT):
                    nkt = 4 * qt + 4  # causal: k-tiles 0..4qt+3

                    # heads share denominator PSUM banks pairwise (partitions
                    # 0 and 64 of one bank) so no per-head den-bank recycling
                    # stall exists; two banks serve all four heads per chunk.
                    dentA = dnp.tile([65, 512], F32, tag="denA", name=f"denA_{qt}")
                    dentB = dnp.tile([65, 512], F32, tag="denB", name=f"denB_{qt}")

                    ctx_q = []  # per-head normalized ctx^T [128, 512] tiles
                    for h in range(HPC):
                        dent = dentA if h < 2 else dentB
                        den = dent[64 * (h % 2):64 * (h % 2) + 1, :]
                        cps = cxp.tile([128, 512], F32, tag="cps", name=f"cps_{h}_{qt}")
                        for kt in range(nkt):
                            j = kt - 4 * qt
                            sc, lo = pend
                            # emit the next scores matmul ahead of this tile's
                            # den/AV so the in-order PE queue always has work
                            # while the exp runs on the Act engine (including
                            # across head and q-chunk boundaries: the next
                            # chunk's first scores land ahead of the out-proj).
                            nxt = next_tile(qt, h, kt)
                            if nxt is not None:
                                pend = emit_scores(*nxt)
                            pt = ptp.tile([128, 512], F16, tag="pt", name=f"pt_{h}_{qt}_{kt}")
                            nc.scalar.activation(
                                pt[:, lo:], sc[:, lo:], EXP, bias=ebias[:], scale=SCALE
                            )
                            if j >= 0:
                                # strictly-diagonal 128x128 sub-block mask
                                nc.vector.tensor_mul(
                                    pt[:, j * 128:(j + 1) * 128],
                                    pt[:, j * 128:(j + 1) * 128],
                                    tri[:],
                                )
                            nc.tensor.matmul(
                                den[:, lo:], (ones_col[:, 0:1]), (pt[:, lo:]),
                                start=(kt == 0), stop=(kt == nkt - 1),
                            )
                            nc.tensor.matmul(
                                cps[:, lo:], (vt[kt][:, h * 128:(h + 1) * 128]), (pt[:, lo:]),
                                start=(kt == 0), stop=(kt == nkt - 1),
                            )
                        recip = rcp.tile([1, 512], F16, tag="recip", name=f"recip_{h}_{qt}")
                        with nc.allow_low_precision("fp16 recip feeds a PE broadcast; plenty for softmax norm"):
                            nc.vector.reciprocal(recip[:], den[:])
                        # broadcast recip across partitions with a rank-1 PE
                        # matmul into an out-proj psum slot (512 cycles; the
                        # GPSIMD partition_broadcast is far slower on real HW)
                        rb = ops.tile([128, 512], F32, tag="po", name=f"rb_{h}_{qt}")
                        nc.tensor.matmul(
                            rb[:], (ones_row[:]), (recip[:]), start=True, stop=True
                        )
                        rcb = rcbp.tile([128, 512], F32, tag="rcb", name=f"rcb_{h}_{qt}")
                        nc.vector.tensor_copy(rcb[:], rb[:])
                        ctx = ctxt[h][qt]
                        nc.vector.tensor_mul(ctx[:], cps[:], rcb[:])
                        ctx_q.append(ctx)

                    # out-projection for this query chunk (4 q-tiles of 128).
                    # po stays one PSUM bank per (r, oc) group, but the four
                    # copies land in one wide SBUF tile so the output row
                    # block leaves in a single fully-contiguous 4KB-row DMA.
                    for r in range(4):
                        q = qt * 4 + r
                        ot = osp.tile([128, 2048], F16, tag="ot", name=f"ot_{q}")
                        last = (qt == QT - 1 and r == 3)
                        for oc in range(4):
                            po = ops.tile([128, 512], F32, tag="po", name=f"po_{oc}_{q}")
                            for i in range(HPC):
                                nc.tensor.matmul(
                                    po[:],
                                    (ctx_q[i][:, r * 128:(r + 1) * 128]),
                                    (wots[(oc, i)][:]),
                                    start=(i == 0),
                                    stop=(i == HPC - 1),
                                )
                            nc.vector.tensor_copy(
                                ot[:, oc * 512:(oc + 1) * 512], po[:]
                            )
                            if last:
                                # stream the final row block per-oc so the
                                # drain tail overlaps the remaining matmuls
                                nc.sync.dma_start(
                                    out=out_d[q * 128:(q + 1) * 128,
                                              oc * 512:(oc + 1) * 512],
                                    in_=ot[:, oc * 512:(oc + 1) * 512],
                                )
                        if not last:
                            nc.sync.dma_start(
                                out=out_d[q * 128:(q + 1) * 128, :],
                                in_=ot[:],
                            )

    nc.compile()
    return nc


def _get_built():
    global _BUILT
    if _BUILT is None:
        _BUILT = _build()
    return _BUILT


def make_in_maps(x, wq, wk, wv, wo):
    x = np.asarray(x, dtype=np.float32)
    wq = np.asarray(wq, dtype=np.float32)
    wk = np.asarray(wk, dtype=np.float32)
    wv = np.asarray(wv, dtype=np.float32)
    wo = np.asarray(wo, dtype=np.float32)
    in_maps = []
    for c in range(NCORES):
        b, hg = divmod(c, NCORES // B)
        sl = slice(hg * DC, (hg + 1) * DC)
        in_maps.append({
            "xT": np.ascontiguousarray(x[b].T).astype(np.float16),
            "wqT": np.ascontiguousarray(wq[sl, :].T).astype(np.float16),
            "wkT": np.ascontiguousarray(wk[sl, :].T).astype(np.float16),
            "wvT": np.ascontiguousarray(wv[sl, :].T).astype(np.float16),
            "woT": np.ascontiguousarray(wo[:, sl].T).astype(np.float16),
        })
    return in_maps


def combine_outputs(results, bo):
    bo = np.asarray(bo, dtype=np.float32)
    out = np.zeros((B, S, D), dtype=np.float32)
    for c in range(NCORES):
        b = c // (NCORES // B)
        out[b] += results[c]["out"].astype(np.float32)
    out += bo[None, None, :]
    return out


def kernel(x, wq, wk, wv, wo, bo):
    nc = _get_built()
    in_maps = make_in_maps(x, wq, wk, wv, wo)
    res = run_bass_kernel_spmd(nc, in_maps, core_ids=list(range(NCORES)))
    return combine_outputs(res.results, bo)


if __name__ == "__main__":
    nc = _get_built()
    print("built ok; instructions:", len(nc.inst_map))


# revision 8
# speedup vs baseline: 1.0482x; 1.0482x over previous
"""Trainium2 Bass kernel for causal multi-head attention (fp16 rewrite).

Problem: B=2, S=2048, D=2048, H=16 heads (HD=128), fp32 in/out, causal.
Sharding: 8 cores = 2 batches (data parallel) x 4 head-groups (tensor
parallel, 4 heads each). Each core computes Q/K/V projections for its
head slice, causal attention, and a partial out-projection; the host
sums the 4 partials per batch (in fp32) and adds the output bias.

Device layout notes:
  - fp16 operands everywhere on the PE (1 cycle/row with NO minimum
    moving-dim threshold, unlike fp32r's N>=256), fp32 PSUM accumulation.
    fp16's 10 mantissa bits keep the global rel err in the few-1e-4 range.
  - All weight tiles persist in SBUF (loaded once, 6 MB fp16); x chunks
    are double-buffered and prefetched so the PE never waits on DMA at
    chunk boundaries.
  - Scores are computed transposed (scores^T[k, q]); softmax runs
    unnormalized with a constant -6 bias folded into the exp so fp16 pt
    can't overflow (exp(s/sqrt(128) - 6) <= ~e); the bias cancels in the
    normalization exactly.
  - Softmax denominators accumulate on the PE via a ones-vector matmul
    (cheapest engine for partition reductions at 2.4 GHz); the
    reciprocal is broadcast across partitions by a rank-1 PE matmul into
    a borrowed out-proj PSUM slot (a GPSIMD partition_broadcast measured
    no faster on real HW and sits on the per-head critical path).
  - Scores matmuls are emitted one step ahead of each tile's den/AV
    matmuls (including across head/chunk boundaries) so the in-order PE
    queue always has work while the exp runs on the Act engine; a dummy
    exp at program start pulls the Act Exp-table load off the phase-2
    critical path.
  - PSUM: phase 2 uses exactly 8 banks: scores x2, ctx x2, den x2,
    out-proj x2. Denominators share banks pairwise (partitions 0/64),
    so no per-head den-bank recycling stall exists. Output row blocks
    leave as one contiguous 4KB-row DMA per q-tile (streamed per-oc for
    the final tile to overlap the drain).
"""

import sys

if "/opt/trn_rl_repo" not in sys.path:
    sys.path.insert(0, "/opt/trn_rl_repo")

import numpy as np

import concourse.bacc as bacc
import concourse.mybir as mybir
import concourse.tile as tile
from concourse.bass_utils import run_bass_kernel_spmd
from concourse.masks import make_upper_triangular

B, S, D, H = 2, 2048, 2048, 16
HD = 128                 # head dim
NCORES = 8
HPC = 4                  # heads per core
DC = HPC * HD            # 512: per-core projection width
CT = D // 128            # 16 contraction tiles
QT = S // 512            # 4 query chunks of 512
ST = S // 128            # 16 seq tiles of 128
SCALE = 1.0 / float(np.sqrt(HD))
EBIAS = -6.0             # constant max-sub shift; cancels in normalization
F32 = mybir.dt.float32
F16 = mybir.dt.float16
EXP = mybir.ActivationFunctionType.Exp

_BUILT = None


def _build(cfg=None, reps=1):
    cfg = cfg or {}
    PTB = cfg.get("ptb", 4)    # p^T tile bufs
    SCB = cfg.get("scb", 2)    # scores psum bufs
    CPB = cfg.get("cpb", 2)    # ctx psum bufs
    DNB = cfg.get("dnb", 1)    # denom psum bufs (per A/B tag: 2 banks total)
    PPB = cfg.get("ppb", 2)    # proj psum bufs
    OPB = cfg.get("opb", 2)    # out-proj psum bufs
    nc = bacc.Bacc(trn_type="TRN2", target_bir_lowering=False)
    xT_d = nc.dram_tensor("xT", [D, S], F16, kind="ExternalInput")
    wqT_d = nc.dram_tensor("wqT", [D, DC], F16, kind="ExternalInput")
    wkT_d = nc.dram_tensor("wkT", [D, DC], F16, kind="ExternalInput")
    wvT_d = nc.dram_tensor("wvT", [D, DC], F16, kind="ExternalInput")
    woT_d = nc.dram_tensor("woT", [DC, D], F16, kind="ExternalInput")
    out_d = nc.dram_tensor("out", [S, D], F16, kind="ExternalOutput")

    with tile.TileContext(nc) as tc:
      for _rep in range(reps):
        _p = f"r{_rep}_"
        with (
            tc.tile_pool(name=_p + "const", bufs=1) as cst,
            tc.tile_pool(name=_p + "persist", bufs=1) as pp,
        ):
            # upper-triangular (incl diagonal) 0/1 mask: allowed = k <= q
            tri = cst.tile([128, 128], F16, tag="tri", name="tri")
            make_upper_triangular(nc, tri[:], val=1.0, diag=True)
            ones_f = cst.tile([128, 1], F32, tag="ones_f", name="ones_f")
            nc.vector.memset(ones_f[:], 1.0)
            ones_col = cst.tile([128, 1], F16, tag="ones_col", name="ones_col")
            nc.vector.tensor_copy(ones_col[:], ones_f[:])
            ebias = cst.tile([128, 1], F32, tag="ebias", name="ebias")
            nc.vector.memset(ebias[:], EBIAS)
            ones_rf = cst.tile([1, 128], F32, tag="ones_rf", name="ones_rf")
            nc.vector.memset(ones_rf[:], 1.0)
            ones_row = cst.tile([1, 128], F16, tag="ones_row", name="ones_row")
            nc.vector.tensor_copy(ones_row[:], ones_rf[:])
            # dummy exp at program start: pulls the Act engine's Exp table
            # load (~1.3us) into phase 1 where Act is idle, instead of the
            # first real softmax tile at the phase-2 critical path.
            warm = cst.tile([1, 2], F16, tag="warm", name="warm")
            nc.scalar.activation(warm[:], ones_rf[:, 0:2], EXP, scale=SCALE)

            # persistent per-core tensors (partition dim x free dim):
            # qT/kT: per head [HD, S]; v: per s-tile [128, DC]; ctx^T per
            # (head, q-chunk) for fine-grained deps so the out-projection of
            # chunk qt can overlap attention of chunk qt+1.
            qTt = [pp.tile([128, S], F16, tag=f"qT{h}", name=f"qT{h}") for h in range(HPC)]
            kTt = [pp.tile([128, S], F16, tag=f"kT{h}", name=f"kT{h}") for h in range(HPC)]
            vt = [pp.tile([128, DC], F16, tag=f"v{s}", name=f"v{s}") for s in range(ST)]
            ctxt = [[pp.tile([128, 512], F16, tag=f"ctx{h}_{q}", name=f"ctx{h}_{q}")
                     for q in range(QT)] for h in range(HPC)]

            # ---------------- Phase 1: Q/K/V projections ----------------
            with (
                tc.tile_pool(name=_p + "wqkv", bufs=1) as wp,
                tc.tile_pool(name=_p + "xc", bufs=2) as xcp,
                tc.tile_pool(name=_p + "proj_psum", bufs=PPB, space="PSUM") as pps,
            ):
                # Load all weight tiles once; interleave chunk-0 x tiles with
                # the wq tiles so the first matmuls' operands arrive first.
                wq_t, wk_t, wv_t = [], [], []
                xcs = []
                for ct in range(CT):
                    xc = xcp.tile([128, 512], F16, tag=f"xc{ct}", name=f"xc_0_{ct}")
                    nc.sync.dma_start(
                        out=xc[:], in_=xT_d[ct * 128:(ct + 1) * 128, 0:512])
                    xcs.append(xc)
                    w = wp.tile([128, DC], F16, tag=f"wq{ct}", name=f"wq{ct}")
                    nc.sync.dma_start(out=w[:], in_=wqT_d[ct * 128:(ct + 1) * 128, :])
                    wq_t.append(w)
                for w_d, lst, nm in ((wkT_d, wk_t, "wk"), (wvT_d, wv_t, "wv")):
                    for ct in range(CT):
                        w = wp.tile([128, DC], F16, tag=f"{nm}{ct}", name=f"{nm}{ct}")
                        nc.sync.dma_start(out=w[:], in_=w_d[ct * 128:(ct + 1) * 128, :])
                        lst.append(w)

                for n in range(QT):  # s-chunks of 512
                    if n + 1 < QT:
                        # prefetch next chunk (double-buffered tags)
                        nxt = []
                        for ct in range(CT):
                            xc = xcp.tile([128, 512], F16, tag=f"xc{ct}",
                                          name=f"xc_{n + 1}_{ct}")
                            nc.sync.dma_start(
                                out=xc[:],
                                in_=xT_d[ct * 128:(ct + 1) * 128,
                                         (n + 1) * 512:(n + 2) * 512])
                            nxt.append(xc)

                    # Q^T and K^T: out[d-tile(=head) 128, s 512] accum over ct
                    for w_t, dst in ((wq_t, qTt), (wk_t, kTt)):
                        acc = [pps.tile([128, 512], F32, tag=f"acc{m}", name=f"acc_{n}_{m}")
                               for m in range(HPC)]
                        for ct in range(CT):
                            for m in range(HPC):
                                nc.tensor.matmul(
                                    acc[m][:],
                                    (w_t[ct][:, m * 128:(m + 1) * 128]),
                                    (xcs[ct][:]),
                                    start=(ct == 0),
                                    stop=(ct == CT - 1),
                                )
                        for m in range(HPC):
                            nc.vector.tensor_copy(
                                dst[m][:, n * 512:(n + 1) * 512], acc[m][:]
                            )

                    # V natural [s-tile 128, d 512]: lhsT = x^T chunk slice
                    accv = [pps.tile([128, 512], F32, tag=f"acc{ss}", name=f"accv_{n}_{ss}")
                            for ss in range(4)]
                    for ct in range(CT):
                        for ss in range(4):
                            nc.tensor.matmul(
                                accv[ss][:],
                                (xcs[ct][:, ss * 128:(ss + 1) * 128]),
                                (wv_t[ct][:]),
                                start=(ct == 0),
                                stop=(ct == CT - 1),
                            )
                    for ss in range(4):
                        nc.vector.tensor_copy(vt[n * 4 + ss][:], accv[ss][:])
                    if n + 1 < QT:
                        xcs = nxt

            # ------- Phase 2+3: causal attention with interleaved out-proj ----
            with (
                tc.tile_pool(name=_p + "ptp", bufs=PTB) as ptp,
                tc.tile_pool(name=_p + "rcp", bufs=2) as rcp,
                tc.tile_pool(name=_p + "rcb", bufs=2) as rcbp,
                tc.tile_pool(name=_p + "wo", bufs=1) as wop,
                tc.tile_pool(name=_p + "osb", bufs=3) as osp,
                tc.tile_pool(name=_p + "out_ps", bufs=OPB, space="PSUM") as ops,
                tc.tile_pool(name=_p + "ctx_ps", bufs=CPB, space="PSUM") as cxp,
                tc.tile_pool(name=_p + "den_ps", bufs=DNB, space="PSUM") as dnp,
                tc.tile_pool(name=_p + "sc_ps", bufs=SCB, space="PSUM") as scp,
            ):
                # preload all out-proj weight tiles (16KB/partition fp16)
                wots = {}
                for oc in range(4):
                    for i in range(HPC):
                        wo_t = wop.tile([128, 512], F16, tag=f"wo{oc}_{i}",
                                        name=f"wo_{oc}_{i}")
                        nc.sync.dma_start(
                            out=wo_t[:],
                            in_=woT_d[i * 128:(i + 1) * 128, oc * 512:(oc + 1) * 512],
                        )
                        wots[(oc, i)] = wo_t

                def emit_scores(qt, h, kt):
                    j = kt - 4 * qt
                    # Diagonal blocks: only q-cols >= 128j are unmasked;
                    # shrink the moving range (fp16 runs 1 cyc/row at any
                    # N, so N=128 is fine).
                    lo = 0 if j < 0 else j * 128
                    sc = scp.tile([128, 512], F32, tag="sc", name=f"sc_{h}_{qt}_{kt}")
                    nc.tensor.matmul(
                        sc[:, lo:],
                        (kTt[h][:, kt * 128:(kt + 1) * 128]),
                        (qTt[h][:, qt * 512 + lo:(qt + 1) * 512]),
                        start=True,
                        stop=True,
                    )
                    return sc, lo

                def next_tile(qt, h, kt):
                    nkt = 4 * qt + 4
                    if kt + 1 < nkt:
                        return (qt, h, kt + 1)
                    if h + 1 < HPC:
                        return (qt, h + 1, 0)
                    if qt + 1 < QT:
                        return (qt + 1, 0, 0)
                    return None

                pend = emit_scores(0, 0, 0)
                for qt in range(QT):
                    nkt = 4 * qt + 4  # causal: k-tiles 0..4qt+3

                    # heads share denominator PSUM banks pairwise (partitions
                    # 0 and 64 of one bank) so no per-head den-bank recycling
                    # stall exists; two banks serve all four heads per chunk.
                    dentA = dnp.tile([65, 512], F32, tag="denA", name=f"denA_{qt}")
                    dentB = dnp.tile([65, 512], F32, tag="denB", name=f"denB_{qt}")

                    ctx_q = []  # per-head normalized ctx^T [128, 512] tiles
                    for h in range(HPC):
                        dent = dentA if h < 2 else dentB
                        den = dent[64 * (h % 2):64 * (h % 2) + 1, :]
                        cps = cxp.tile([128, 512], F32, tag="cps", name=f"cps_{h}_{qt}")
                        for kt in range(nkt):
                            j = kt - 4 * qt
                            sc, lo = pend
                            # emit the next scores matmul ahead of this tile's
                            # den/AV so the in-order PE queue always has work
                            # while the exp runs on the Act engine (including
                            # across head and q-chunk boundaries: the next
                            # chunk's first scores land ahead of the out-proj).
                            nxt = next_tile(qt, h, kt)
                            if nxt is not None:
                                pend = emit_scores(*nxt)
                            pt = ptp.tile([128, 512], F16, tag="pt", name=f"pt_{h}_{qt}_{kt}")
                            nc.scalar.activation(
                                pt[:, lo:], sc[:, lo:], EXP, bias=ebias[:], scale=SCALE
                            )
                            if j >= 0:
                                # strictly-diagonal 128x128 sub-block mask
                                nc.vector.tensor_mul(
                                    pt[:, j * 128:(j + 1) * 128],
                                    pt[:, j * 128:(j + 1) * 128],
                                    tri[:],
                                )
                            nc.tensor.matmul(
                                den[:, lo:], (ones_col[:, 0:1]), (pt[:, lo:]),
                                start=(kt == 0), stop=(kt == nkt - 1),
                            )
                            nc.tensor.matmul(
                                cps[:, lo:], (vt[kt][:, h * 128:(h + 1) * 128]), (pt[:, lo:]),
                                start=(kt == 0), stop=(kt == nkt - 1),
                            )
                        recip = rcp.tile([1, 512], F16, tag="recip", name=f"recip_{h}_{qt}")
                        with nc.allow_low_precision("fp16 recip feeds a PE broadcast; plenty for softmax norm"):
                            nc.vector.reciprocal(recip[:], den[:])
                        # broadcast recip across partitions with a rank-1 PE
                        # matmul into an out-proj psum slot (512 cycles; the
                        # GPSIMD partition_broadcast is far slower on real HW)
                        rb = ops.tile([128, 512], F32, tag="po", name=f"rb_{h}_{qt}")
                        nc.tensor.matmul(
                            rb[:], (ones_row[:]), (recip[:]), start=True, stop=True
                        )
                        rcb = rcbp.tile([128, 512], F32, tag="rcb", name=f"rcb_{h}_{qt}")
                        nc.vector.tensor_copy(rcb[:], rb[:])
                        ctx = ctxt[h][qt]
                        nc.vector.tensor_mul(ctx[:], cps[:], rcb[:])
                        ctx_q.append(ctx)

                    # out-projection for this query chunk (4 q-tiles of 128).
                    # po stays one PSUM bank per (r, oc) group, but the four
                    # copies land in one wide SBUF tile so the output row
                    # block leaves in a single fully-contiguous 4KB-row DMA.
                    for r in range(4):
                        q = qt * 4 + r
                        ot = osp.tile([128, 2048], F16, tag="ot", name=f"ot_{q}")
                        last = (qt == QT - 1 and r == 3)
                        for oc in range(4):
                            po = ops.tile([128, 512], F32, tag="po", name=f"po_{oc}_{q}")
                            for i in range(HPC):
                                nc.tensor.matmul(
                                    po[:],
                                    (ctx_q[i][:, r * 128:(r + 1) * 128]),
                                    (wots[(oc, i)][:]),
                                    start=(i == 0),
                                    stop=(i == HPC - 1),
                                )
                            nc.vector.tensor_copy(
                                ot[:, oc * 512:(oc + 1) * 512], po[:]
                            )
                            if last:
                                # stream the final row block per-oc so the
                                # drain tail overlaps the remaining matmuls
                                nc.sync.dma_start(
                                    out=out_d[q * 128:(q + 1) * 128,
                                              oc * 512:(oc + 1) * 512],
                                    in_=ot[:, oc * 512:(oc + 1) * 512],
                                )
                        if not last:
                            nc.sync.dma_start(
                                out=out_d[q * 128:(q + 1) * 128, :],
                                in_=ot[:],
                            )

    nc.compile()
    return nc


def _get_built():
    global _BUILT
    if _BUILT is None:
        _BUILT = _build()
    return _BUILT


def make_in_maps(x, wq, wk, wv, wo):
    x = np.asarray(x, dtype=np.float32)
    wq = np.asarray(wq, dtype=np.float32)
    wk = np.asarray(wk, dtype=np.float32)
    wv = np.asarray(wv, dtype=np.float32)
    wo = np.asarray(wo, dtype=np.float32)
    in_maps = []
    for c in range(NCORES):
        b, hg = divmod(c, NCORES // B)
        sl = slice(hg * DC, (hg + 1) * DC)
        in_maps.append({
            "xT": np.ascontiguousarray(x[b].T).astype(np.float16),
            "wqT": np.ascontiguousarray(wq[sl, :].T).astype(np.float16),
            "wkT": np.ascontiguousarray(wk[sl, :].T).astype(np.float16),
            "wvT": np.ascontiguousarray(wv[sl, :].T).astype(np.float16),
            "woT": np.ascontiguousarray(wo[:, sl].T).astype(np.float16),
        })
    return in_maps


def combine_outputs(results, bo):
    bo = np.asarray(bo, dtype=np.float32)
    out = np.zeros((B, S, D), dtype=np.float32)
    for c in range(NCORES):
        b = c // (NCORES // B)
        out[b] += results[c]["out"].astype(np.float32)
    out += bo[None, None, :]
    return out


def kernel(x, wq, wk, wv, wo, bo):
    nc = _get_built()
    in_maps = make_in_maps(x, wq, wk, wv, wo)
    res = run_bass_kernel_spmd(nc, in_maps, core_ids=list(range(NCORES)))
    return combine_outputs(res.results, bo)


if __name__ == "__main__":
    nc = _get_built()
    print("built ok; instructions:", len(nc.inst_map))


# revision 10
# speedup vs baseline: 1.0868x; 1.0369x over previous
"""Trainium2 Bass kernel for causal multi-head attention (fp16 rewrite).

Problem: B=2, S=2048, D=2048, H=16 heads (HD=128), fp32 in/out, causal.
Sharding: 8 cores = 2 batches (data parallel) x 4 head-groups (tensor
parallel, 4 heads each). Each core computes Q/K/V projections for its
head slice, causal attention, and a partial out-projection; the host
sums the 4 partials per batch (in fp32) and adds the output bias.

Device layout notes:
  - fp16 operands everywhere on the PE (1 cycle/row with NO minimum
    moving-dim threshold, unlike fp32r's N>=256), fp32 PSUM accumulation.
    fp16's 10 mantissa bits keep the global rel err in the few-1e-4 range.
  - All weight tiles persist in SBUF (loaded once, 6 MB fp16); x chunks
    are double-buffered and prefetched so the PE never waits on DMA at
    chunk boundaries.
  - Scores are computed transposed (scores^T[k, q]); softmax runs
    unnormalized with a constant -6 bias folded into the exp so fp16 pt
    can't overflow (exp(s/sqrt(128) - 6) <= ~e); the bias cancels in the
    normalization exactly.
  - Softmax denominators accumulate on the PE via a ones-vector matmul
    (cheapest engine for partition reductions at 2.4 GHz); the
    reciprocal is broadcast across partitions by a rank-1 PE matmul into
    a borrowed out-proj PSUM slot (a GPSIMD partition_broadcast measured
    no faster on real HW and sits on the per-head critical path).
  - Scores matmuls are emitted TWO tiles ahead of each tile's den/AV
    matmuls (including across head/chunk boundaries) so the in-order PE
    queue always has work while the exp runs on the Act engine; a dummy
    exp at program start pulls the Act Exp-table load off the phase-2
    critical path.
  - PSUM: phase 2 uses exactly 8 banks: scores x3, ctx x2, den x1,
    out-proj x2. All four heads of a chunk share the one den bank
    (pairs at partitions 0/64, second pair reuses after the first
    pair's reciprocals); the freed bank funds the third scores buffer.
    Output row blocks leave as one contiguous 4KB-row DMA per q-tile
    (streamed per-oc for the final tile to overlap the drain).
"""

import sys

if "/opt/trn_rl_repo" not in sys.path:
    sys.path.insert(0, "/opt/trn_rl_repo")

import numpy as np

import concourse.bacc as bacc
import concourse.mybir as mybir
import concourse.tile as tile
from concourse.bass_utils import run_bass_kernel_spmd
from concourse.masks import make_upper_triangular

B, S, D, H = 2, 2048, 2048, 16
HD = 128                 # head dim
NCORES = 8
HPC = 4                  # heads per core
DC = HPC * HD            # 512: per-core projection width
CT = D // 128            # 16 contraction tiles
QT = S // 512            # 4 query chunks of 512
ST = S // 128            # 16 seq tiles of 128
SCALE = 1.0 / float(np.sqrt(HD))
EBIAS = -6.0             # constant max-sub shift; cancels in normalization
F32 = mybir.dt.float32
F16 = mybir.dt.float16
EXP = mybir.ActivationFunctionType.Exp

_BUILT = None


def _build(cfg=None, reps=1):
    cfg = cfg or {}
    PTB = cfg.get("ptb", 4)    # p^T tile bufs
    SCB = cfg.get("scb", 3)    # scores psum bufs
    CPB = cfg.get("cpb", 2)    # ctx psum bufs
    DNB = cfg.get("dnb", 1)    # denom psum bufs (per A/B tag: 2 banks total)
    PPB = cfg.get("ppb", 2)    # proj psum bufs
    OPB = cfg.get("opb", 2)    # out-proj psum bufs
    nc = bacc.Bacc(trn_type="TRN2", target_bir_lowering=False)
    xT_d = nc.dram_tensor("xT", [D, S], F16, kind="ExternalInput")
    wqT_d = nc.dram_tensor("wqT", [D, DC], F16, kind="ExternalInput")
    wkT_d = nc.dram_tensor("wkT", [D, DC], F16, kind="ExternalInput")
    wvT_d = nc.dram_tensor("wvT", [D, DC], F16, kind="ExternalInput")
    woT_d = nc.dram_tensor("woT", [DC, D], F16, kind="ExternalInput")
    out_d = nc.dram_tensor("out", [S, D], F16, kind="ExternalOutput")

    with tile.TileContext(nc) as tc:
      for _rep in range(reps):
        _p = f"r{_rep}_"
        with (
            tc.tile_pool(name=_p + "const", bufs=1) as cst,
            tc.tile_pool(name=_p + "persist", bufs=1) as pp,
        ):
            # upper-triangular (incl diagonal) 0/1 mask: allowed = k <= q
            tri = cst.tile([128, 128], F16, tag="tri", name="tri")
            make_upper_triangular(nc, tri[:], val=1.0, diag=True)
            ones_f = cst.tile([128, 1], F32, tag="ones_f", name="ones_f")
            nc.vector.memset(ones_f[:], 1.0)
            ones_col = cst.tile([128, 1], F16, tag="ones_col", name="ones_col")
            nc.vector.tensor_copy(ones_col[:], ones_f[:])
            ebias = cst.tile([128, 1], F32, tag="ebias", name="ebias")
            nc.vector.memset(ebias[:], EBIAS)
            ones_rf = cst.tile([1, 128], F32, tag="ones_rf", name="ones_rf")
            nc.vector.memset(ones_rf[:], 1.0)
            ones_row = cst.tile([1, 128], F16, tag="ones_row", name="ones_row")
            nc.vector.tensor_copy(ones_row[:], ones_rf[:])
            # dummy exp at program start: pulls the Act engine's Exp table
            # load (~1.3us) into phase 1 where Act is idle, instead of the
            # first real softmax tile at the phase-2 critical path.
            warm = cst.tile([1, 2], F16, tag="warm", name="warm")
            nc.scalar.activation(warm[:], ones_rf[:, 0:2], EXP, scale=SCALE)

            # persistent per-core tensors (partition dim x free dim):
            # qT/kT: per head [HD, S]; v: per s-tile [128, DC]; ctx^T per
            # (head, q-chunk) for fine-grained deps so the out-projection of
            # chunk qt can overlap attention of chunk qt+1.
            qTt = [pp.tile([128, S], F16, tag=f"qT{h}", name=f"qT{h}") for h in range(HPC)]
            kTt = [pp.tile([128, S], F16, tag=f"kT{h}", name=f"kT{h}") for h in range(HPC)]
            vt = [pp.tile([128, DC], F16, tag=f"v{s}", name=f"v{s}") for s in range(ST)]
            ctxt = [[pp.tile([128, 512], F16, tag=f"ctx{h}_{q}", name=f"ctx{h}_{q}")
                     for q in range(QT)] for h in range(HPC)]

            # ---------------- Phase 1: Q/K/V projections ----------------
            with (
                tc.tile_pool(name=_p + "wqkv", bufs=1) as wp,
                tc.tile_pool(name=_p + "xc", bufs=2) as xcp,
                tc.tile_pool(name=_p + "proj_psum", bufs=PPB, space="PSUM") as pps,
            ):
                # Load all weight tiles once; interleave chunk-0 x tiles with
                # the wq tiles so the first matmuls' operands arrive first.
                wq_t, wk_t, wv_t = [], [], []
                xcs = []
                for ct in range(CT):
                    xc = xcp.tile([128, 512], F16, tag=f"xc{ct}", name=f"xc_0_{ct}")
                    nc.sync.dma_start(
                        out=xc[:], in_=xT_d[ct * 128:(ct + 1) * 128, 0:512])
                    xcs.append(xc)
                    w = wp.tile([128, DC], F16, tag=f"wq{ct}", name=f"wq{ct}")
                    nc.sync.dma_start(out=w[:], in_=wqT_d[ct * 128:(ct + 1) * 128, :])
                    wq_t.append(w)
                for w_d, lst, nm in ((wkT_d, wk_t, "wk"), (wvT_d, wv_t, "wv")):
                    for ct in range(CT):
                        w = wp.tile([128, DC], F16, tag=f"{nm}{ct}", name=f"{nm}{ct}")
                        nc.sync.dma_start(out=w[:], in_=w_d[ct * 128:(ct + 1) * 128, :])
                        lst.append(w)

                for n in range(QT):  # s-chunks of 512
                    if n + 1 < QT:
                        # prefetch next chunk (double-buffered tags)
                        nxt = []
                        for ct in range(CT):
                            xc = xcp.tile([128, 512], F16, tag=f"xc{ct}",
                                          name=f"xc_{n + 1}_{ct}")
                            nc.sync.dma_start(
                                out=xc[:],
                                in_=xT_d[ct * 128:(ct + 1) * 128,
                                         (n + 1) * 512:(n + 2) * 512])
                            nxt.append(xc)

                    # Q^T and K^T: out[d-tile(=head) 128, s 512] accum over ct
                    for w_t, dst in ((wq_t, qTt), (wk_t, kTt)):
                        acc = [pps.tile([128, 512], F32, tag=f"acc{m}", name=f"acc_{n}_{m}")
                               for m in range(HPC)]
                        for ct in range(CT):
                            for m in range(HPC):
                                nc.tensor.matmul(
                                    acc[m][:],
                                    (w_t[ct][:, m * 128:(m + 1) * 128]),
                                    (xcs[ct][:]),
                                    start=(ct == 0),
                                    stop=(ct == CT - 1),
                                )
                        for m in range(HPC):
                            nc.vector.tensor_copy(
                                dst[m][:, n * 512:(n + 1) * 512], acc[m][:]
                            )

                    # V natural [s-tile 128, d 512]: lhsT = x^T chunk slice
                    accv = [pps.tile([128, 512], F32, tag=f"acc{ss}", name=f"accv_{n}_{ss}")
                            for ss in range(4)]
                    for ct in range(CT):
                        for ss in range(4):
                            nc.tensor.matmul(
                                accv[ss][:],
                                (xcs[ct][:, ss * 128:(ss + 1) * 128]),
                                (wv_t[ct][:]),
                                start=(ct == 0),
                                stop=(ct == CT - 1),
                            )
                    for ss in range(4):
                        nc.vector.tensor_copy(vt[n * 4 + ss][:], accv[ss][:])
                    if n + 1 < QT:
                        xcs = nxt

            # ------- Phase 2+3: causal attention with interleaved out-proj ----
            with (
                tc.tile_pool(name=_p + "ptp", bufs=PTB) as ptp,
                tc.tile_pool(name=_p + "rcp", bufs=2) as rcp,
                tc.tile_pool(name=_p + "rcb", bufs=2) as rcbp,
                tc.tile_pool(name=_p + "wo", bufs=1) as wop,
                tc.tile_pool(name=_p + "osb", bufs=3) as osp,
                tc.tile_pool(name=_p + "out_ps", bufs=OPB, space="PSUM") as ops,
                tc.tile_pool(name=_p + "ctx_ps", bufs=CPB, space="PSUM") as cxp,
                tc.tile_pool(name=_p + "den_ps", bufs=DNB, space="PSUM") as dnp,
                tc.tile_pool(name=_p + "sc_ps", bufs=SCB, space="PSUM") as scp,
            ):
                # preload all out-proj weight tiles (16KB/partition fp16)
                wots = {}
                for oc in range(4):
                    for i in range(HPC):
                        wo_t = wop.tile([128, 512], F16, tag=f"wo{oc}_{i}",
                                        name=f"wo_{oc}_{i}")
                        nc.sync.dma_start(
                            out=wo_t[:],
                            in_=woT_d[i * 128:(i + 1) * 128, oc * 512:(oc + 1) * 512],
                        )
                        wots[(oc, i)] = wo_t

                def emit_scores(qt, h, kt):
                    j = kt - 4 * qt
                    # Diagonal blocks: only q-cols >= 128j are unmasked;
                    # shrink the moving range (fp16 runs 1 cyc/row at any
                    # N, so N=128 is fine).
                    lo = 0 if j < 0 else j * 128
                    sc = scp.tile([128, 512], F32, tag="sc", name=f"sc_{h}_{qt}_{kt}")
                    nc.tensor.matmul(
                        sc[:, lo:],
                        (kTt[h][:, kt * 128:(kt + 1) * 128]),
                        (qTt[h][:, qt * 512 + lo:(qt + 1) * 512]),
                        start=True,
                        stop=True,
                    )
                    return sc, lo

                def next_tile(qt, h, kt):
                    nkt = 4 * qt + 4
                    if kt + 1 < nkt:
                        return (qt, h, kt + 1)
                    if h + 1 < HPC:
                        return (qt, h + 1, 0)
                    if qt + 1 < QT:
                        return (qt + 1, 0, 0)
                    return None

                pend = [emit_scores(0, 0, 0)]
                pend_pos = [(0, 0, 0)]
                nx = next_tile(0, 0, 0)
                if nx is not None:
                    pend.append(emit_scores(*nx))
                    pend_pos.append(nx)
                for qt in range(QT):
                    nkt = 4 * qt + 4  # causal: k-tiles 0..4qt+3

                    # all four heads share ONE denominator PSUM bank: heads
                    # of a pair sit at partitions 0/64; the second pair reuses
                    # the bank after the first pair's reciprocals read it.
                    # This frees a bank for a third scores buffer (SCB=3),
                    # deepening the scores lookahead to 2 tiles.
                    ctx_q = []  # per-head normalized ctx^T [128, 512] tiles
                    for h in range(HPC):
                        if h % 2 == 0:
                            dent = dnp.tile([65, 512], F32, tag="den",
                                            name=f"den_{qt}_{h // 2}")
                        den = dent[64 * (h % 2):64 * (h % 2) + 1, :]
                        cps = cxp.tile([128, 512], F32, tag="cps", name=f"cps_{h}_{qt}")
                        for kt in range(nkt):
                            j = kt - 4 * qt
                            sc, lo = pend.pop(0)
                            pend_pos.pop(0)
                            # emit scores two tiles ahead of this tile's
                            # den/AV so the in-order PE queue always has work
                            # while the exp runs on the Act engine (including
                            # across head and q-chunk boundaries: the next
                            # chunk's first scores land ahead of the out-proj).
                            nxt = next_tile(*pend_pos[-1]) if pend_pos else None
                            if nxt is not None:
                                pend.append(emit_scores(*nxt))
                                pend_pos.append(nxt)
                            pt = ptp.tile([128, 512], F16, tag="pt", name=f"pt_{h}_{qt}_{kt}")
                            nc.scalar.activation(
                                pt[:, lo:], sc[:, lo:], EXP, bias=ebias[:], scale=SCALE
                            )
                            if j >= 0:
                                # strictly-diagonal 128x128 sub-block mask
                                nc.vector.tensor_mul(
                                    pt[:, j * 128:(j + 1) * 128],
                                    pt[:, j * 128:(j + 1) * 128],
                                    tri[:],
                                )
                            nc.tensor.matmul(
                                den[:, lo:], (ones_col[:, 0:1]), (pt[:, lo:]),
                                start=(kt == 0), stop=(kt == nkt - 1),
                            )
                            nc.tensor.matmul(
                                cps[:, lo:], (vt[kt][:, h * 128:(h + 1) * 128]), (pt[:, lo:]),
                                start=(kt == 0), stop=(kt == nkt - 1),
                            )
                        recip = rcp.tile([1, 512], F16, tag="recip", name=f"recip_{h}_{qt}")
                        with nc.allow_low_precision("fp16 recip feeds a PE broadcast; plenty for softmax norm"):
                            nc.vector.reciprocal(recip[:], den[:])
                        # broadcast recip across partitions with a rank-1 PE
                        # matmul into an out-proj psum slot (512 cycles; the
                        # GPSIMD partition_broadcast is far slower on real HW)
                        rb = ops.tile([128, 512], F32, tag="po", name=f"rb_{h}_{qt}")
                        nc.tensor.matmul(
                            rb[:], (ones_row[:]), (recip[:]), start=True, stop=True
                        )
                        rcb = rcbp.tile([128, 512], F32, tag="rcb", name=f"rcb_{h}_{qt}")
                        nc.vector.tensor_copy(rcb[:], rb[:])
                        ctx = ctxt[h][qt]
                        nc.vector.tensor_mul(ctx[:], cps[:], rcb[:])
                        ctx_q.append(ctx)

                    # out-projection for this query chunk (4 q-tiles of 128).
                    # po stays one PSUM bank per (r, oc) group, but the four
                    # copies land in one wide SBUF tile so the output row
                    # block leaves in a single fully-contiguous 4KB-row DMA.
                    for r in range(4):
                        q = qt * 4 + r
                        ot = osp.tile([128, 2048], F16, tag="ot", name=f"ot_{q}")
                        last = (qt == QT - 1 and r == 3)
                        for oc in range(4):
                            po = ops.tile([128, 512], F32, tag="po", name=f"po_{oc}_{q}")
                            for i in range(HPC):
                                nc.tensor.matmul(
                                    po[:],
                                    (ctx_q[i][:, r * 128:(r + 1) * 128]),
                                    (wots[(oc, i)][:]),
                                    start=(i == 0),
                                    stop=(i == HPC - 1),
                                )
                            nc.vector.tensor_copy(
                                ot[:, oc * 512:(oc + 1) * 512], po[:]
                            )
                            if last:
                                # stream the final row block per-oc so the
                                # drain tail overlaps the remaining matmuls
                                nc.sync.dma_start(
                                    out=out_d[q * 128:(q + 1) * 128,
                                              oc * 512:(oc + 1) * 512],
                                    in_=ot[:, oc * 512:(oc + 1) * 512],
                                )
                        if not last:
                            nc.sync.dma_start(
                                out=out_d[q * 128:(q + 1) * 128, :],
                                in_=ot[:],
                            )

    nc.compile()
    return nc


def _get_built():
    global _BUILT
    if _BUILT is None:
        _BUILT = _build()
    return _BUILT


def make_in_maps(x, wq, wk, wv, wo):
    x = np.asarray(x, dtype=np.float32)
    wq = np.asarray(wq, dtype=np.float32)
    wk = np.asarray(wk, dtype=np.float32)
    wv = np.asarray(wv, dtype=np.float32)
    wo = np.asarray(wo, dtype=np.float32)
    in_maps = []
    for c in range(NCORES):
        b, hg = divmod(c, NCORES // B)
        sl = slice(hg * DC, (hg + 1) * DC)
        in_maps.append({
            "xT": np.ascontiguousarray(x[b].T).astype(np.float16),
            "wqT": np.ascontiguousarray(wq[sl, :].T).astype(np.float16),
            "wkT": np.ascontiguousarray(wk[sl, :].T).astype(np.float16),
            "wvT": np.ascontiguousarray(wv[sl, :].T).astype(np.float16),
            "woT": np.ascontiguousarray(wo[:, sl].T).astype(np.float16),
        })
    return in_maps


def combine_outputs(results, bo):
    bo = np.asarray(bo, dtype=np.float32)
    out = np.zeros((B, S, D), dtype=np.float32)
    for c in range(NCORES):
        b = c // (NCORES // B)
        out[b] += results[c]["out"].astype(np.float32)
    out += bo[None, None, :]
    return out


def kernel(x, wq, wk, wv, wo, bo):
    nc = _get_built()
    in_maps = make_in_maps(x, wq, wk, wv, wo)
    res = run_bass_kernel_spmd(nc, in_maps, core_ids=list(range(NCORES)))
    return combine_outputs(res.results, bo)


if __name__ == "__main__":
    nc = _get_built()
    print("built ok; instructions:", len(nc.inst_map))
